# revision 1
# baseline (speedup 1.0000x reference)
import numpy as np
import jax
import jax.numpy as jnp
from functools import partial

# nn_AttentionPairBiasPairformerDeepspeed — 8-core sequence-parallel kernel.
# Shapes (hardcoded per spec): B=1, L=768, c_a=384, c_pair=128, H=16, c=24.
# Sharding: i-axis (first residue axis) split 8 ways -> 96 rows of i per core.
# Weights + A_I replicated; each core computes its i-slice of the output.

LN_EPS = 1e-5
L = 768
NCORES = 8
LS = L // NCORES  # 96


def _layernorm(x, gamma, beta):
    mu = jnp.mean(x, axis=-1, keepdims=True)
    var = jnp.mean(jnp.square(x - mu), axis=-1, keepdims=True)
    return (x - mu) * jax.lax.rsqrt(var + LN_EPS) * gamma + beta


@partial(jax.pmap, axis_name="i",
         in_axes=(0, 0, None, None, None, None, None, None, None, None, None, None, None))
def _shard_fn(Z_s, Beta_s, A, Wq, Wk, Wv, Wg, Wb, Wa, ln0_g, ln0_b, ln1_g, ln1_b):
    # Z_s: [LS, L, c_pair], Beta_s: [LS, L], A: [L, c_a]
    c = Wq.shape[-1]
    idx = jax.lax.axis_index("i")
    bf = jnp.bfloat16
    a = _layernorm(A, ln1_g, ln1_b).astype(bf)              # [L, c_a]
    a_i = jax.lax.dynamic_slice_in_dim(a, idx * LS, LS, 0)  # [LS, c_a]
    scale = jnp.asarray(1.0 / np.sqrt(c), bf)
    q = jnp.einsum("id,dhc->ihc", a_i, Wq.astype(bf)) * scale   # [LS,H,c]
    k = jnp.einsum("jd,dhc->jhc", a, Wk.astype(bf))             # [L,H,c]
    v = jnp.einsum("jd,dhc->jhc", a, Wv.astype(bf))             # [L,H,c]
    g = jax.nn.sigmoid(jnp.einsum("id,dhc->ihc", a_i, Wg.astype(bf)))  # [LS,H,c]
    b_ijh = jnp.einsum("ijd,dh->ijh", _layernorm(Z_s, ln0_g, ln0_b), Wb) \
        + Beta_s[..., None]                                      # [LS,L,H] fp32
    logits = jnp.einsum("ihd,jhd->ijh", q, k).astype(jnp.float32) + b_ijh
    attn = jax.nn.softmax(logits, axis=-2)                       # softmax over j
    o = jnp.einsum("ijh,jhc->ihc", attn, v.astype(jnp.float32))  # [LS,H,c]
    o = (g.astype(jnp.float32) * o).reshape(LS, -1)              # [LS,c_a]
    return jnp.einsum("id,de->ie", o, Wa)                        # [LS,c_a]


def kernel(A_I, Z_II, Beta_II, Wq, Wk, Wv, Wg, Wb, Wa, ln0_g, ln0_b, ln1_g, ln1_b):
    A = np.asarray(A_I)[0]                      # [L, c_a]
    Z_sh = np.asarray(Z_II)[0].reshape(NCORES, LS, L, Z_II.shape[-1])
    B_sh = np.asarray(Beta_II)[0].reshape(NCORES, LS, L)
    out = _shard_fn(Z_sh, B_sh, A,
                    np.asarray(Wq), np.asarray(Wk), np.asarray(Wv),
                    np.asarray(Wg), np.asarray(Wb), np.asarray(Wa),
                    np.asarray(ln0_g), np.asarray(ln0_b),
                    np.asarray(ln1_g), np.asarray(ln1_b))
    out = np.asarray(out).reshape(1, L, -1).astype(np.float32)
    return out


# revision 3
# speedup vs baseline: 11.1223x; 11.1223x over previous
import numpy as np
import jax
import jax.numpy as jnp
from functools import partial

# nn_AttentionPairBiasPairformerDeepspeed — 8-core sequence-parallel kernel.
# Shapes (hardcoded per spec): B=1, L=768, c_a=384, c_pair=128, H=16, c=24.
# Sharding: i-axis (first residue axis) split 8 ways -> 96 rows of i per core.
# Weights + A_I replicated; each core computes its i-slice of the output.

LN_EPS = 1e-5
L = 768
NCORES = 8
LS = L // NCORES  # 96


def _layernorm(x, gamma, beta):
    mu = jnp.mean(x, axis=-1, keepdims=True)
    var = jnp.mean(jnp.square(x - mu), axis=-1, keepdims=True)
    return (x - mu) * jax.lax.rsqrt(var + LN_EPS) * gamma + beta


@partial(jax.pmap, axis_name="i",
         in_axes=(0, 0, None, None, None, None, None, None, None, None, None, None, None))
def _shard_fn(Z_s, Beta_s, A, Wq, Wk, Wv, Wg, Wb, Wa, ln0_g, ln0_b, ln1_g, ln1_b):
    # Z_s: [LS, L, c_pair] bf16 (transferred compressed), Beta_s: [LS, L], A: [L, c_a]
    Z_s = Z_s.astype(jnp.float32)
    c = Wq.shape[-1]
    idx = jax.lax.axis_index("i")
    bf = jnp.bfloat16
    a = _layernorm(A, ln1_g, ln1_b).astype(bf)              # [L, c_a]
    a_i = jax.lax.dynamic_slice_in_dim(a, idx * LS, LS, 0)  # [LS, c_a]
    scale = jnp.asarray(1.0 / np.sqrt(c), bf)
    q = jnp.einsum("id,dhc->ihc", a_i, Wq.astype(bf)) * scale   # [LS,H,c]
    k = jnp.einsum("jd,dhc->jhc", a, Wk.astype(bf))             # [L,H,c]
    v = jnp.einsum("jd,dhc->jhc", a, Wv.astype(bf))             # [L,H,c]
    g = jax.nn.sigmoid(jnp.einsum("id,dhc->ihc", a_i, Wg.astype(bf)))  # [LS,H,c]
    b_ijh = jnp.einsum("ijd,dh->ijh", _layernorm(Z_s, ln0_g, ln0_b), Wb) \
        + Beta_s[..., None]                                      # [LS,L,H] fp32
    logits = jnp.einsum("ihd,jhd->ijh", q, k).astype(jnp.float32) + b_ijh
    attn = jax.nn.softmax(logits, axis=-2)                       # softmax over j
    o = jnp.einsum("ijh,jhc->ihc", attn, v.astype(jnp.float32))  # [LS,H,c]
    o = (g.astype(jnp.float32) * o).reshape(LS, -1)              # [LS,c_a]
    return jnp.einsum("id,de->ie", o, Wa)                        # [LS,c_a]


def kernel(A_I, Z_II, Beta_II, Wq, Wk, Wv, Wg, Wb, Wa, ln0_g, ln0_b, ln1_g, ln1_b):
    import ml_dtypes
    A = np.asarray(A_I)[0]                      # [L, c_a]
    Z_sh = np.asarray(Z_II)[0].reshape(NCORES, LS, L, Z_II.shape[-1])
    Z_sh = Z_sh.astype(ml_dtypes.bfloat16)      # halve host->device bytes
    B_sh = np.asarray(Beta_II)[0].reshape(NCORES, LS, L)
    out = _shard_fn(Z_sh, B_sh, A,
                    np.asarray(Wq), np.asarray(Wk), np.asarray(Wv),
                    np.asarray(Wg), np.asarray(Wb), np.asarray(Wa),
                    np.asarray(ln0_g), np.asarray(ln0_b),
                    np.asarray(ln1_g), np.asarray(ln1_b))
    out = np.asarray(out).reshape(1, L, -1).astype(np.float32)
    return out


# revision 5
# speedup vs baseline: 231.5403x; 20.8177x over previous
import numpy as np
import jax
import jax.numpy as jnp
from functools import partial

# nn_AttentionPairBiasPairformerDeepspeed — 8-core sequence-parallel kernel.
# Shapes (hardcoded per spec): B=1, L=768, c_a=384, c_pair=128, H=16, c=24.
# Sharding: i-axis (first residue axis) split 8 ways -> 96 rows of i per core.
# Weights + A_I replicated; each core computes its i-slice of the output.

LN_EPS = 1e-5
L = 768
NCORES = 8
LS = L // NCORES  # 96


def _layernorm(x, gamma, beta):
    mu = jnp.mean(x, axis=-1, keepdims=True)
    var = jnp.mean(jnp.square(x - mu), axis=-1, keepdims=True)
    return (x - mu) * jax.lax.rsqrt(var + LN_EPS) * gamma + beta


def _shard_body(Z_s, Beta_s, A, Wq, Wk, Wv, Wg, Wb, Wa, ln0_g, ln0_b, ln1_g, ln1_b):
    # Z_s: [LS, L, c_pair] bf16 (transferred compressed), Beta_s: [LS, L], A: [L, c_a]
    Z_s = Z_s.astype(jnp.float32)
    c = Wq.shape[-1]
    idx = jax.lax.axis_index("i")
    bf = jnp.bfloat16
    a = _layernorm(A, ln1_g, ln1_b).astype(bf)              # [L, c_a]
    a_i = jax.lax.dynamic_slice_in_dim(a, idx * LS, LS, 0)  # [LS, c_a]
    scale = jnp.asarray(1.0 / np.sqrt(c), bf)
    q = jnp.einsum("id,dhc->ihc", a_i, Wq.astype(bf)) * scale   # [LS,H,c]
    k = jnp.einsum("jd,dhc->jhc", a, Wk.astype(bf))             # [L,H,c]
    v = jnp.einsum("jd,dhc->jhc", a, Wv.astype(bf))             # [L,H,c]
    g = jax.nn.sigmoid(jnp.einsum("id,dhc->ihc", a_i, Wg.astype(bf)))  # [LS,H,c]
    b_ijh = jnp.einsum("ijd,dh->ijh", _layernorm(Z_s, ln0_g, ln0_b), Wb) \
        + Beta_s[..., None]                                      # [LS,L,H] fp32
    logits = jnp.einsum("ihd,jhd->ijh", q, k).astype(jnp.float32) + b_ijh
    attn = jax.nn.softmax(logits, axis=-2)                       # softmax over j
    o = jnp.einsum("ijh,jhc->ihc", attn, v.astype(jnp.float32))  # [LS,H,c]
    o = (g.astype(jnp.float32) * o).reshape(LS, -1)              # [LS,c_a]
    return jnp.einsum("id,de->ie", o, Wa)                        # [LS,c_a]


_shard_fn = partial(jax.pmap, axis_name="i",
                    in_axes=(0, 0) + (None,) * 11)(_shard_body)


def kernel(A_I, Z_II, Beta_II, Wq, Wk, Wv, Wg, Wb, Wa, ln0_g, ln0_b, ln1_g, ln1_b):
    import ml_dtypes
    A = np.asarray(A_I)[0]                      # [L, c_a]
    Z_sh = np.asarray(Z_II)[0].reshape(NCORES, LS, L, Z_II.shape[-1])
    Z_sh = Z_sh.astype(ml_dtypes.bfloat16)      # halve host->device bytes
    B_sh = np.asarray(Beta_II)[0].reshape(NCORES, LS, L)
    out = _shard_fn(Z_sh, B_sh, A,
                    np.asarray(Wq), np.asarray(Wk), np.asarray(Wv),
                    np.asarray(Wg), np.asarray(Wb), np.asarray(Wa),
                    np.asarray(ln0_g), np.asarray(ln0_b),
                    np.asarray(ln1_g), np.asarray(ln1_b))
    out = np.asarray(out).reshape(1, L, -1).astype(np.float32)
    return out
